# revision 13
# baseline (speedup 1.0000x reference)
"""NTM head addressing kernel for Trainium2 (8 NeuronCores, data-parallel over heads).

Shapes (hardcoded): B=4096 heads, N=2048 memory rows, C=128 memory cols.
Each core processes 512 heads as 4 tiles of 128 (partition dim = head).

Math restructuring vs the reference (exact up to fp rounding):
  - w = w_tilde^gamma / sum(w_tilde^gamma) is invariant to any per-head
    positive scale on w_tilde.  Drop the softmax normalizer of s (divide
    taps by s1), and fold the interpolation gate into the exp bias:
        e2  = exp(beta'*sim + g_raw - 4ln2)     (per-head positive scale,
                                                 absorbed by normalization)
        b   = sum(e2) * exp(-g_raw)
        u   = b*w_prev + e2
        v_j = s0'*u_{j-1} + u_j + s2'*u_{j+1}   (circular, s0'=e^{s0-s1},
                                                 s2'=e^{s2-s1})
        w   = v^gamma' / sum(v^gamma')
  - Output-shifted conv: V_j = v_{j+1} = s0'*u_j + u_{j+1} + s2'*u_{j+2};
    the final DMA writes the output columns shifted by +1 (circular).
  - conv3(w_prev) is precomputed on HOST (input-only transform):
        cwp_j = s0'*wp_j + wp_{j+1} + s2'*wp_{j+2}
    so the whole w_prev contribution enters as one per-head scale:
        V = s0'*e + e_{+1} + s2'*e_{+2} + b*cwp
  - beta' is folded into kT on host (columns scaled per head).

Engine schedule (ACT is the critical engine and the floor, ~6.6us/tile):
  ACT:  e-exp (fp16 out, fused row-sum; tile 0 in halves so it trails the
        MT DMA), ln straight out of PSUM, y-exp (bf16 out, fused sum;
        tile 3 in halves to shorten the tail).  ~100% busy after e0.
  DVE:  t12 = s0'*e + b*cwp as two 4x-mode tensor_scalar ops + one 2x
        tensor_tensor (short ~2.9us latency after each e), then
        wout = y/sumy, and tiny glue.
  PE:   heaters (memset source) warm the HAM throttle to 2.4GHz before
        the logits; per tile 4 logits matmuls and the 3-term PSUM
        accumulation pv = I@e_{+1} + diag(s2')@e_{+2} + I@t12; junk
        LDWEIGHTS (lowest priority, keyed to y tiles) fill idle gaps so
        the PE never re-throttles.  logits1 goes through the pv slot to
        remove the e0->e1 seam (lg and pv are the whole PSUM).
  DMA:  sc/kT/MT/eye+d2 first, cwp deferred (MT gets full bandwidth),
        outputs stream per tile with the +1 circular shift.
"""

import os
import numpy as np

_B, _N, _C = 4096, 2048, 128
_NCORES = 8
_BS = _B // _NCORES      # 512 heads per core
_NT = _BS // 128         # 4 head tiles per core

_HEATERS = int(os.environ.get("NTM_HEATERS", "40"))
_LDW_SPAM = int(os.environ.get("NTM_LDW_SPAM", "12"))

_built = None

_ONE_SET = "natural_log_exp_and_others"
_PINNED = {"Exp", "Ln", "Square", "Copy", "Identity"}


def _patch_act_tables():
    """Force Exp/Ln/Square/Copy onto the one table set that holds them all,
    so bacc's load inserter cannot thrash between per-function sets."""
    import concourse.bacc as bacc
    import concourse.hw_specs as hw_specs
    import concourse.mybir as mybir

    if getattr(bacc, "_ntm_table_patch", False):
        return
    orig = hw_specs.get_activation_tables
    pinned = {
        getattr(mybir.ActivationFunctionType, n)
        for n in _PINNED
        if hasattr(mybir.ActivationFunctionType, n)
    }

    def patched(module_arch):
        tables = orig(module_arch)
        out = {}
        for name, fns in tables.items():
            if name != _ONE_SET:
                fns = fns - pinned
            out[name] = fns
        return out

    bacc.get_activation_tables = patched
    bacc._ntm_table_patch = True


def _build():
    """Construct the (SPMD, per-core) Bass program."""
    import concourse.bass as bass
    import concourse.bacc as bacc
    import concourse.mybir as mybir
    import concourse.tile as tile

    _patch_act_tables()

    f32 = mybir.dt.float32
    bf16 = mybir.dt.bfloat16
    f16 = mybir.dt.float16
    AF = mybir.ActivationFunctionType
    OP = mybir.AluOpType

    nc = bacc.Bacc(
        "TRN2", target_bir_lowering=False, debug=False, num_devices=_NCORES
    )
    sc_d = nc.declare_dram_parameter("sc", [128, _NT * 5], f32, isOutput=False)
    kT_d = nc.declare_dram_parameter("kT", [_C, _BS], bf16, isOutput=False)
    MT_d = nc.declare_dram_parameter("MT", [_C, _N], bf16, isOutput=False)
    # eye [128] | diag(s2') per tile [4*128]
    ed_d = nc.declare_dram_parameter("ed", [128, 128 + _NT * 128], f16, isOutput=False)
    cwp0_d = nc.declare_dram_parameter("cwp0", [128, _N], f16, isOutput=False)
    cwp123_d = nc.declare_dram_parameter(
        "cwp123", [128, 3 * _N], f16, isOutput=False
    )
    out_d = nc.declare_dram_parameter("out", [_BS, _N], bf16, isOutput=True)

    with tile.TileContext(nc) as tc:
        with (
            tc.tile_pool(name="const", bufs=1) as constp,
            tc.tile_pool(name="slab", bufs=2) as slabp,
            tc.tile_pool(name="mini", bufs=2) as minip,
            tc.tile_pool(name="psum", bufs=1, space=bass.MemorySpace.PSUM) as psump,
        ):
            # ------- head-critical input DMAs (sync queue order) ----------
            sc = constp.tile([128, _NT * 5], f32)
            nc.sync.dma_start(sc[:], sc_d[:])
            kT = constp.tile([_C, _BS], bf16)
            nc.scalar.dma_start(kT[:], kT_d[:])
            MT = constp.tile([_C, _N], bf16)
            nc.sync.dma_start(MT[:, 0:1024], MT_d[:][:, 0:1024])
            nc.scalar.dma_start(MT[:, 1024:_N], MT_d[:][:, 1024:_N])
            ed = constp.tile([128, 128 + _NT * 128], f16)
            nc.sync.dma_start(ed[:], ed_d[:])
            eye = ed[:, 0:128]
            # cwp DMAs are deferred (emitted later) so MT gets the bandwidth

            # dummy activation so the one ACT table load happens during the
            # DMA fill instead of right before the first real exp
            junk = minip.tile([128, 1], f32, tag="junk")
            nc.gpsimd.memset(junk[:], 1.0)
            nc.scalar.activation(junk[:], junk[:], AF.Exp)

            # heater stock: memset tile, available before any DMA lands
            jmm = constp.tile([128, 128], bf16, tag="jmm")
            nc.gpsimd.memset(jmm[:], 0.5)

            # scalar column blocks
            graw = sc[:, 0:_NT]                      # biased: graw - 4ln2
            eginv = sc[:, _NT : 2 * _NT]             # exp(-graw)
            gprime = sc[:, 2 * _NT : 3 * _NT]
            s0p = sc[:, 3 * _NT : 4 * _NT]
            s2p = sc[:, 4 * _NT : 5 * _NT]

            # PSUM: lg (4 banks) + pv (4 banks) = the whole PSUM
            lg = psump.tile([128, _N], f32, tag="lg", bufs=1, name="lg")
            pv = psump.tile([128, _N], f32, tag="pv", bufs=1, name="pv")

            def emit_heaters(n):
                for _ in range(n):
                    nc.tensor.matmul(
                        pv[:, 0:128], jmm[:], jmm[:],
                        start=True, stop=True, skip_group_check=True,
                    )

            def emit_logits(t, piece, dst=None):
                d = lg if dst is None else dst
                for c0 in range(0, _N, piece):
                    nc.tensor.matmul(
                        d[:, c0 : c0 + piece],
                        kT[:, t * 128 : (t + 1) * 128],
                        MT[:, c0 : c0 + piece],
                    )

            ebufs, bs, seps = {}, {}, {}

            def emit_e(t, src_psum=None, halved=False):
                """Full-width exp of the logits (beta' folded into kT) with
                fused row-sum; optionally two halves (trails the MT DMA)."""
                src = (src_psum if src_psum is not None else lg)[:]
                e = slabp.tile([128, _N + 2], f16, tag="e", bufs=4, name=f"e{t}")
                if halved:
                    sp = minip.tile([128, 2], f32, tag=f"sep{t}", name=f"sep{t}")
                    for h in range(2):
                        nc.scalar.activation(
                            e[:, h * 1024 : (h + 1) * 1024],
                            src[:, h * 1024 : (h + 1) * 1024],
                            AF.Exp, bias=graw[:, t : t + 1],
                            accum_out=sp[:, h : h + 1],
                        )
                    b = minip.tile([128, 1], f32, tag=f"b{t}", name=f"b{t}")
                    nc.vector.tensor_scalar(
                        b[:], sp[:, 0:1], sp[:, 1:2], eginv[:, t : t + 1],
                        OP.add, OP.mult,
                    )
                    bs[t] = b
                else:
                    sep = minip.tile([128, 1], f32, tag=f"sep{t}", name=f"sep{t}")
                    nc.scalar.activation(
                        e[:, 0:_N], src, AF.Exp,
                        bias=graw[:, t : t + 1], accum_out=sep[:],
                    )
                    seps[t] = sep
                ebufs[t] = e

            def emit_b(t):
                b = minip.tile([128, 1], f32, tag=f"b{t}", name=f"b{t}")
                nc.vector.tensor_scalar_mul(b[:], seps[t][:], eginv[:, t : t + 1])
                bs[t] = b

            def emit_wrap(t):
                nc.vector.tensor_copy(ebufs[t][:, _N : _N + 2], ebufs[t][:, 0:2])

            def emit_chain(t, cwp_ap):
                """t12 = s0'*e + b*cwp on DVE (short critical path; the
                shifted taps ride the PE)."""
                e = ebufs[t]
                a1 = slabp.tile([128, _N], f16, tag="a1", bufs=2, name=f"a1_{t}")
                nc.vector.tensor_scalar_mul(a1[:], e[:, 0:_N], s0p[:, t : t + 1])
                a3 = slabp.tile([128, _N], f16, tag="a3", bufs=2, name=f"a3_{t}")
                nc.vector.tensor_scalar_mul(a3[:], cwp_ap, bs[t][:])
                t12 = slabp.tile([128, _N], f16, tag="t12", bufs=2, name=f"t12_{t}")
                nc.vector.tensor_tensor(t12[:], a1[:], a3[:], OP.add)
                return t12

            def emit_shift_taps(t, dst):
                """dst = I @ e_{+1} + diag(s2') @ e_{+2} (accumulation open)."""
                e = ebufs[t]
                d2 = ed[:, 128 + t * 128 : 128 + (t + 1) * 128]
                for q in range(4):
                    sl = slice(q * 512, (q + 1) * 512)
                    nc.tensor.matmul(
                        dst[:, sl], eye, e[:, 1 : _N + 1][:, sl],
                        start=True, stop=False, skip_group_check=True,
                    )
                    nc.tensor.matmul(
                        dst[:, sl], d2, e[:, 2 : _N + 2][:, sl],
                        start=False, stop=False, skip_group_check=True,
                    )

            def emit_chain_full(t, cwp_ap):
                """Whole V on DVE (tail tiles): no PE taps, no pv slot —
                ln reads the SBUF f16 result directly.  Latency is long but
                these chains run far ahead of their ln slots."""
                e = ebufs[t]
                a1 = slabp.tile([128, _N], f16, tag="a1", bufs=2, name=f"a1_{t}")
                nc.vector.tensor_scalar_mul(a1[:], e[:, 0:_N], s0p[:, t : t + 1])
                a2 = slabp.tile([128, _N], f16, tag="a2", bufs=2, name=f"a2_{t}")
                nc.vector.tensor_scalar_mul(a2[:], e[:, 2 : _N + 2], s2p[:, t : t + 1])
                t1 = slabp.tile([128, _N], f16, tag="t1", bufs=2, name=f"t1_{t}")
                nc.vector.tensor_tensor(t1[:], a1[:], a2[:], OP.add)
                a3 = slabp.tile([128, _N], f16, tag="a3", bufs=2, name=f"a3_{t}")
                nc.vector.tensor_scalar_mul(a3[:], cwp_ap, bs[t][:])
                t2 = slabp.tile([128, _N], f16, tag="t2", bufs=2, name=f"t2_{t}")
                nc.vector.tensor_tensor(t2[:], t1[:], a3[:], OP.add)
                v = slabp.tile([128, _N], f16, tag="v", bufs=2, name=f"v{t}")
                nc.vector.tensor_tensor(v[:], t2[:], e[:, 1 : _N + 1], OP.add)
                return v

            def emit_t12_taps(t, t12, dst):
                """dst += I @ t12 (closes the accumulation group)."""
                for q in range(4):
                    sl = slice(q * 512, (q + 1) * 512)
                    nc.tensor.matmul(
                        dst[:, sl], eye, t12[:, sl],
                        start=False, stop=True, skip_group_check=True,
                    )

            ys, sumys = {}, {}

            def emit_ln(t, src=None):
                lw = slabp.tile([128, _N], f32, tag="lw", bufs=2, name=f"lw{t}")
                nc.scalar.activation(lw[:], (pv if src is None else src)[:], AF.Ln)
                return lw

            def emit_y(t, lw, halved=False):
                y = slabp.tile([128, _N], bf16, tag="y", bufs=4, name=f"y{t}")
                if halved:
                    sp = minip.tile([128, 2], f32, tag=f"syp{t}", name=f"syp{t}")
                    for h in range(2):
                        nc.scalar.activation(
                            y[:, h * 1024 : (h + 1) * 1024],
                            lw[:, h * 1024 : (h + 1) * 1024],
                            AF.Exp, scale=gprime[:, t : t + 1],
                            accum_out=sp[:, h : h + 1],
                        )
                    sumy = minip.tile([128, 1], f32, tag=f"sumy{t}", name=f"sumy{t}")
                    nc.vector.tensor_add(sumy[:], sp[:, 0:1], sp[:, 1:2])
                else:
                    sumy = minip.tile([128, 1], f32, tag=f"sumy{t}", name=f"sumy{t}")
                    nc.scalar.activation(
                        y[:], lw[:], AF.Exp,
                        scale=gprime[:, t : t + 1], accum_out=sumy[:],
                    )
                ys[t] = y
                sumys[t] = sumy

            def emit_tail(t, halves=False):
                """wout = y/sumy (TS 4x on bf16); output DMA with the +1
                circular column shift."""
                r = minip.tile([128, 1], f32, tag=f"r{t}", name=f"r{t}")
                nc.vector.reciprocal(r[:], sumys[t][:])
                wout = slabp.tile([128, _N], bf16, tag="wout", bufs=2, name=f"wout{t}")
                rows = slice(t * 128, (t + 1) * 128)
                if halves:
                    nc.vector.tensor_scalar_mul(wout[:, 0:1024], ys[t][:, 0:1024], r[:])
                    nc.sync.dma_start(out_d[:][rows, 1:1025], wout[:, 0:1024])
                    nc.vector.tensor_scalar_mul(
                        wout[:, 1024:_N], ys[t][:, 1024:_N], r[:]
                    )
                    nc.sync.dma_start(out_d[:][rows, 1025:_N], wout[:, 1024 : _N - 1])
                    nc.sync.dma_start(out_d[:][rows, 0:1], wout[:, _N - 1 : _N])
                else:
                    nc.vector.tensor_scalar_mul(wout[:], ys[t][:], r[:])
                    nc.sync.dma_start(out_d[:][rows, 1:_N], wout[:, 0 : _N - 1])
                    nc.sync.dma_start(out_d[:][rows, 0:1], wout[:, _N - 1 : _N])

            def emit_ldw_spam(src_ap, n):
                """Junk LDWEIGHTS: become ready when their source tile does
                and lose priority ties to real work, so the scheduler uses
                them to fill PE idle gaps (keeps the HAM throttle warm)."""
                for i in range(n):
                    nc.tensor.ldweights(src_ap)

            # --------- emission order = per-engine queue priorities -------
            # PSUM slot plan:
            #   lg: logits0 -> e0 -> taps0 -> ln0 -> taps1 -> ln1 -> free
            #   pv: heaters -> logits1 -> e1 -> logits2 -> e2 -> logits3 -> e3
            emit_heaters(_HEATERS)          # PE: busy from ~6.5us (into pv)
            emit_logits(0, 128)             # PE -> lg: trail the MT DMA
            emit_e(0)                       # ACT
            # deferred cwp DMAs (split across the two HWDGE rings)
            cwp0 = constp.tile([128, _N], f16, tag="cwp0", name="cwp0")
            nc.sync.dma_start(cwp0[:], cwp0_d[:])
            cwp123 = constp.tile([128, 3 * _N], f16, tag="cwp123", name="cwp123")
            for t in range(3):
                nc.scalar.dma_start(
                    cwp123[:, t * _N : (t + 1) * _N],
                    cwp123_d[:][:, t * _N : (t + 1) * _N],
                )
            cwp_aps = [
                cwp0[:],
                cwp123[:, 0:_N],
                cwp123[:, _N : 2 * _N],
                cwp123[:, 2 * _N : 3 * _N],
            ]
            emit_logits(1, 512, dst=pv)     # PE (pv free after heaters)
            emit_ldw_spam(jmm[:], 30)       # PE: bridge logits1 -> taps0
            emit_b(0)                       # DVE
            emit_wrap(0)                    # DVE
            t12_0 = emit_chain(0, cwp_aps[0])  # DVE
            emit_shift_taps(0, lg)          # PE (right after e0 reads lg)
            emit_t12_taps(0, t12_0, lg)     # PE
            emit_e(1, src_psum=pv)          # ACT
            emit_logits(2, 512, dst=pv)     # PE (after e1 reads pv)
            emit_b(1)
            emit_wrap(1)
            t12_1 = emit_chain(1, cwp_aps[1])  # DVE
            emit_e(2, src_psum=pv)          # ACT
            lw0 = emit_ln(0, src=lg)        # ACT
            emit_logits(3, 512, dst=pv)     # PE (after e2 reads pv)
            emit_y(0, lw0)                  # ACT
            emit_shift_taps(1, lg)          # PE (after ln0 reads lg)
            emit_t12_taps(1, t12_1, lg)     # PE
            emit_b(2)
            emit_wrap(2)
            v2t = emit_chain_full(2, cwp_aps[2])  # DVE
            emit_e(3, src_psum=pv)          # ACT
            lw1 = emit_ln(1, src=lg)        # ACT
            emit_y(1, lw1)                  # ACT
            emit_b(3)
            emit_wrap(3)
            v3t = emit_chain_full(3, cwp_aps[3])  # DVE
            lw2 = emit_ln(2, src=v2t)       # ACT (SBUF src)
            emit_y(2, lw2)                  # ACT
            emit_tail(0)                    # DVE + sync-ring DMA
            emit_tail(1)
            lw3 = emit_ln(3, src=v3t)       # ACT (SBUF src)
            emit_y(3, lw3, halved=True)     # ACT
            emit_tail(2)
            emit_tail(3, halves=True)
            # lowest-priority PE fillers (scheduler slots them into gaps)
            emit_ldw_spam(ebufs[2][:, 0:128], 50)
            emit_ldw_spam(ys[0][:, 0:128], 12)
            emit_ldw_spam(ys[1][:, 0:128], 12)

    nc.compile()
    return nc


def _get_nc():
    global _built
    if _built is None:
        _built = _build()
    return _built


def _softplus(x):
    return np.log1p(np.exp(np.minimum(x, 30.0))) + np.maximum(x - 30.0, 0.0)


def _make_in_maps(k, beta, g, s, gamma, w_prev, M):
    import ml_dtypes

    bf16 = ml_dtypes.bfloat16
    k = np.asarray(k, dtype=np.float32)
    M = np.asarray(M, dtype=np.float32)
    # host precompute (input-only transforms)
    mnorm = np.sqrt(np.sum(M.astype(np.float64) ** 2, axis=1))
    MTn = np.ascontiguousarray((M / mnorm[:, None].astype(np.float32)).T.astype(bf16))
    knorm = np.sqrt(np.sum(k.astype(np.float64) ** 2, axis=1)).astype(np.float32)
    bprime = (_softplus(beta[:, 0]) / knorm).astype(np.float32)     # [B]
    graw = np.asarray(g[:, 0], dtype=np.float32)
    # scale e2 by 2^-4 so f16 intermediates stay in range; absorbed by the
    # final normalization
    graw_b = graw - 4.0 * np.float32(np.log(2.0))
    gprime = (1.0 + _softplus(gamma[:, 0])).astype(np.float32)
    s0p = np.exp(s[:, 0] - s[:, 1]).astype(np.float32)
    s2p = np.exp(s[:, 2] - s[:, 1]).astype(np.float32)
    eginv = np.exp(-graw).astype(np.float32)

    # beta' folded into kT: per-head column scale
    kTs_full = (k * bprime[:, None]).T  # [C, B] f32

    # conv3 of w_prev with the per-head taps, shifted by +1 (host, f16):
    #   cwp_j = s0'*wp_j + wp_{j+1} + s2'*wp_{j+2}
    wp = np.asarray(w_prev, dtype=np.float32)
    cwp = (
        s0p[:, None] * wp
        + np.roll(wp, -1, axis=1)
        + s2p[:, None] * np.roll(wp, -2, axis=1)
    ).astype(np.float16)

    in_maps = []
    for c in range(_NCORES):
        sl = slice(c * _BS, (c + 1) * _BS)

        def cols(x):
            return np.ascontiguousarray(
                np.asarray(x[sl]).reshape(_NT, 128).T, dtype=np.float32
            )
        sc = np.concatenate(
            [cols(graw_b), cols(eginv), cols(gprime), cols(s0p), cols(s2p)],
            axis=1,
        )
        # eye | per-tile diag(s2')
        ed = np.zeros((128, 128 + _NT * 128), dtype=np.float16)
        ed[:, 0:128] = np.eye(128, dtype=np.float16)
        s2t = np.asarray(s2p[sl]).reshape(_NT, 128)
        for t in range(_NT):
            ed[np.arange(128), 128 + t * 128 + np.arange(128)] = s2t[t].astype(
                np.float16
            )
        im = {
            "sc": np.ascontiguousarray(sc),
            "kT": np.ascontiguousarray(kTs_full[:, sl].astype(bf16)),
            "MT": MTn,
            "ed": np.ascontiguousarray(ed),
            "cwp0": np.ascontiguousarray(cwp[c * _BS : c * _BS + 128]),
            "cwp123": np.ascontiguousarray(
                np.concatenate(
                    [
                        cwp[c * _BS + t * 128 : c * _BS + (t + 1) * 128]
                        for t in range(1, _NT)
                    ],
                    axis=1,
                )
            ),
        }
        in_maps.append(im)
    return in_maps


def kernel(k, beta, g, s, gamma, w_prev, M, _trace=False, _tmpdir=None):
    from concourse.bass_utils import run_bass_kernel_spmd

    nc = _get_nc()
    in_maps = _make_in_maps(
        np.asarray(k), np.asarray(beta), np.asarray(g), np.asarray(s),
        np.asarray(gamma), np.asarray(w_prev), np.asarray(M),
    )
    res = run_bass_kernel_spmd(
        nc, in_maps, list(range(_NCORES)), trace=_trace, tmpdir=_tmpdir
    )
    out = np.concatenate(
        [np.asarray(res.results[c]["out"]).astype(np.float32) for c in range(_NCORES)],
        axis=0,
    )
    if _trace:
        kernel._last_results = res
    return out
